# revision 1
# baseline (speedup 1.0000x reference)
"""Neural CDE kernel for Trainium2 (8 NeuronCores, data-parallel over batch).

Problem shapes (hardcoded per contract): B=512, T=1024, D=8, H=64, W=128.

Host side: knot index / frac from ts (exact fp32 accumulation semantics),
spline derivative dX, initial MLP y0, and folding of dt plus the
tanh(z) = 1 - 2*sigmoid(-2z) rewrite into a padded dX9 tensor.

Device side (per core, 64 samples, scan fully unrolled):
  p1 = Wf0 @ y            (PE, weight-stationary)
  h1 = ln(1 + exp(p1+b0)) (ACT Exp + Ln(bias=1))   [natural_log_exp set]
  p2 = Wf1 @ h1           (PE)
  h2 = ln(1 + exp(p2+b1)) (ACT)
  z  = Wf2 @ h2 + b2      (PE, data-stationary, + K=1 ones-matmul for bias)
  S  = sigmoid(-2z) = exp(-ln(1+exp(2z)))          (ACT x3)
  q[s,h] = sum_d S9[s,(h,d)] * dX9[s,k,d]          (DVE mul + grouped reduce)
           where S9 has a constant-1 column at d=8 and
           dX9[...,d<8] = -2*dt*dX, dX9[...,8] = dt*sum_d dX
           => q = dt * sum_d tanh(z_d) * dX_d
  y += q^T                (PE transpose + DVE add)
  ro[:,k] = y^T @ Wl      (PE, N=1 matmul into accumulating PSUM bank)
Final: sigmoid via the same exp/ln chain, DMA out.
"""

import numpy as np

B, T, D, H, W = 512, 1024, 8, 64, 128
NCORES = 8
S = B // NCORES  # samples per core = 64
D9 = D + 1       # padded derivative cols


# ----------------------------------------------------------------- host math
def _host_precompute(ts, cd, cc, cb, ca, Wi0, bi0, Wi1, bi1, Wi2, bi2):
    f32 = np.float32
    ts = np.asarray(ts, f32)
    dt = (ts[:, 1] - ts[:, 0]).astype(f32)  # (B,)

    # t0 series: t0_{k+1} = t0_k + dt accumulated in fp32 (cumsum is sequential)
    incs = np.concatenate([ts[:, :1], np.tile(dt[:, None], (1, T - 1))], axis=1)
    t0 = np.cumsum(incs, axis=1, dtype=f32)  # (B, T)

    # knot index + frac per row (searchsorted 'right' like the oracle)
    idx = np.empty((B, T), np.int64)
    for b in range(B):
        idx[b] = np.searchsorted(ts[b], t0[b], side="right") - 1
    idx = np.clip(idx, 0, T - 2)
    frac = (t0 - np.take_along_axis(ts, idx, axis=1)).astype(f32)  # (B, T)

    rows = np.arange(B)[:, None]
    cbg = cb[rows, idx]  # (B, T, D)
    ccg = cc[rows, idx]
    cdg = cd[rows, idx]
    fr = frac[:, :, None]
    dX = (cbg + fr * (f32(2.0) * ccg + f32(3.0) * fr * cdg)).astype(f32)

    dtb = dt[:, None, None]
    dX9 = np.empty((B, T, D9), f32)
    dX9[:, :, :D] = f32(-2.0) * dtb * dX
    dX9[:, :, D] = (dtb[:, :, 0] * dX.sum(axis=2)).astype(f32)

    # initial MLP (relu hidden): y0 = Wi2 @ relu(Wi1 @ relu(Wi0 @ a0 + bi0) + bi1) + bi2
    a0 = np.asarray(ca, f32)[:, 0, :]  # (B, D)
    hh = np.maximum(a0 @ np.asarray(Wi0, f32).T + bi0, 0)
    hh = np.maximum(hh @ np.asarray(Wi1, f32).T + bi1, 0)
    y0 = (hh @ np.asarray(Wi2, f32).T + bi2).astype(f32)  # (B, H)
    return dX9, y0


# --------------------------------------------------------------- bass kernel
def _build_kernel(bl_val):
    import concourse.bass as bass
    import concourse.bacc as bacc
    import concourse.mybir as mybir
    from concourse.tile import TileContext

    f32 = mybir.dt.float32
    bf16 = mybir.dt.bfloat16
    AF = mybir.ActivationFunctionType
    ALU = mybir.AluOpType

    nc = bacc.Bacc("TRN2")

    # DRAM I/O (per-core shapes)
    d_w0t = nc.dram_tensor("w0t", [H, W], f32, kind="ExternalInput")      # Wf0^T
    d_w1t = nc.dram_tensor("w1t", [W, W], f32, kind="ExternalInput")      # Wf1^T
    d_w2t = nc.dram_tensor("w2t", [W, H * D], f32, kind="ExternalInput")  # Wf2^T
    d_wlt = nc.dram_tensor("wlt", [H, 1], f32, kind="ExternalInput")      # Wl^T
    d_b0 = nc.dram_tensor("b0", [W, 1], f32, kind="ExternalInput")
    d_b1 = nc.dram_tensor("b1", [W, 1], f32, kind="ExternalInput")
    d_b2 = nc.dram_tensor("b2", [1, H * D], f32, kind="ExternalInput")
    d_ones = nc.dram_tensor("ones1", [1, S], f32, kind="ExternalInput")
    d_ident = nc.dram_tensor("ident", [S, S], f32, kind="ExternalInput")
    d_dx9 = nc.dram_tensor("dx9", [S, T * D9], bf16, kind="ExternalInput")
    d_y0t = nc.dram_tensor("y0t", [H, S], f32, kind="ExternalInput")
    d_out = nc.dram_tensor("out", [S, T], f32, kind="ExternalOutput")

    UNROLL = 16
    assert T % UNROLL == 0

    with TileContext(nc) as tc:
        with (
            tc.tile_pool(name="const", bufs=1) as cpool,
            tc.tile_pool(name="state", bufs=1) as spool,
            tc.tile_pool(name="work", bufs=2) as wpool,
            tc.tile_pool(name="ps", bufs=2, space="PSUM") as ppool,
            tc.tile_pool(name="ps1", bufs=1, space="PSUM") as p1pool,
        ):
            # constants
            w0t = cpool.tile([H, W], f32, tag="w0t")
            w1t = cpool.tile([W, W], f32, tag="w1t")
            w2t = cpool.tile([W, H * D], f32, tag="w2t")
            wlt = cpool.tile([H, 1], f32, tag="wlt")
            b0 = cpool.tile([W, 1], f32, tag="b0")
            b1 = cpool.tile([W, 1], f32, tag="b1")
            b2 = cpool.tile([1, H * D], f32, tag="b2")
            ones1 = cpool.tile([1, S], f32, tag="ones1")
            ident = cpool.tile([S, S], f32, tag="ident")
            dx9 = cpool.tile([S, T * D9], bf16, tag="dx9")
            for dst, src in [
                (w0t, d_w0t), (w1t, d_w1t), (w2t, d_w2t), (wlt, d_wlt),
                (b0, d_b0), (b1, d_b1), (b2, d_b2), (ones1, d_ones),
                (ident, d_ident), (dx9, d_dx9),
            ]:
                nc.gpsimd.dma_start(dst[:], src[:])

            # state
            y = spool.tile([H, S], f32, tag="y")  # (h, s)
            nc.gpsimd.dma_start(y[:], d_y0t[:])
            # S9 double buffer, const-1 column at d=8
            s9 = [
                spool.tile([S, H * D9], bf16, tag=f"s9_{i}", name=f"s9_{i}")
                for i in range(2)
            ]
            for t_ in s9:
                v = t_[:].rearrange("s (h d) -> s h d", d=D9)
                nc.vector.memset(v[:, :, D : D + 1], 1.0)

            ro_sb = spool.tile([S, T], f32, tag="ro_sb")
            ro_ps = p1pool.tile([S, UNROLL], f32, tag="ro_ps")

            # Constants settle before any compute touches them: a matmul
            # (S3_LW struct) cannot carry multiple HWDGE sem waits.
            tc.strict_bb_all_engine_barrier()

            with tc.For_i(0, T // UNROLL, 1) as iv:
              ibase = iv * (UNROLL * D9)
              for j in range(UNROLL):
                k = j  # static within the unrolled body
                s9k = s9[k % 2]
                # ---- mm1: p1 = Wf0 @ y  -> (W, S)
                p1 = ppool.tile([W, S], f32, tag="p12")
                nc.tensor.matmul(p1[:], w0t[:], y[:], start=True, stop=True)
                # ---- softplus 1 (with bias b0 folded into Exp)
                u1 = wpool.tile([W, S], f32, tag="u1")
                h1 = wpool.tile([W, S], f32, tag="h1")
                nc.scalar.activation(u1[:], p1[:], AF.Exp, bias=b0[:])
                nc.scalar.activation(h1[:], u1[:], AF.Ln, bias=1.0)
                # ---- mm2
                p2 = ppool.tile([W, S], f32, tag="p12")
                nc.tensor.matmul(p2[:], w1t[:], h1[:], start=True, stop=True)
                u2 = wpool.tile([W, S], f32, tag="u2")
                h2 = wpool.tile([W, S], f32, tag="h2")
                nc.scalar.activation(u2[:], p2[:], AF.Exp, bias=b1[:])
                nc.scalar.activation(h2[:], u2[:], AF.Ln, bias=1.0)
                # ---- mm3: z = h2^T W2T + b2 -> (S, H*D)
                vf = ppool.tile([S, H * D], f32, tag="vf")
                nc.tensor.matmul(vf[:], ones1[:], b2[:], start=True, stop=False)
                nc.tensor.matmul(vf[:], h2[:], w2t[:], start=False, stop=True)
                # ---- S = sigmoid(-2z) = exp(-ln(1+exp(2z)))
                e2 = wpool.tile([S, H * D], f32, tag="e2")
                l2 = wpool.tile([S, H * D], f32, tag="l2")
                nc.scalar.activation(e2[:], vf[:], AF.Exp, scale=2.0)
                nc.scalar.activation(l2[:], e2[:], AF.Ln, bias=1.0)
                s9v = s9k[:].rearrange("s (h d) -> s h d", d=D9)
                l2v = l2[:].rearrange("s (h d) -> s h d", d=D)
                nc.scalar.activation(s9v[:, :, 0:D], l2v, AF.Exp, scale=-1.0)
                # ---- q[s,h] = sum_d S9 * dX9  (broadcast dx over h)
                m1 = wpool.tile([S, H * D9], bf16, tag="m1")
                dxk = dx9[:, bass.ds(ibase + j * D9, D9)]
                dxb = dxk.rearrange("s (o d) -> s o d", o=1)
                m1v = m1[:].rearrange("s (h d) -> s h d", d=D9)
                s9vv = s9k[:].rearrange("s (h d) -> s h d", d=D9)
                in0b, in1b = bass.broadcast_tensor_aps(s9vv, dxb)
                nc.vector.tensor_tensor(m1v, in0b, in1b, ALU.mult)
                q = wpool.tile([S, H], f32, tag="q")
                nc.vector.tensor_reduce(
                    q[:], m1v, axis=mybir.AxisListType.X, op=ALU.add
                )
                # ---- y += q^T
                qt = ppool.tile([H, S], f32, tag="qt")
                nc.tensor.transpose(qt[:], q[:], ident[:])
                nc.vector.tensor_tensor(y[:], y[:], qt[:], ALU.add)
                # ---- readout column
                nc.tensor.matmul(
                    ro_ps[:, j : j + 1], y[:], wlt[:], start=True, stop=True
                )
                if j == UNROLL - 1:
                    nc.vector.tensor_copy(
                        ro_sb[:, bass.ds(iv * UNROLL, UNROLL)], ro_ps[:]
                    )

            # ---- final sigmoid(v + bl) = exp(-ln(1+exp(-v-bl)))
            eo = spool.tile([S, T], f32, tag="eo")
            nc.scalar.activation(eo[:], ro_sb[:], AF.Exp, scale=-1.0,
                                 bias=float(-bl_val))
            nc.scalar.activation(eo[:], eo[:], AF.Ln, bias=1.0)
            nc.scalar.activation(eo[:], eo[:], AF.Exp, scale=-1.0)
            nc.sync.dma_start(d_out[:], eo[:])

    nc.compile()
    return nc


_NC_CACHE = {}
LAST_RESULTS = None


def _get_nc(bl_val):
    key = float(bl_val)
    if key not in _NC_CACHE:
        _NC_CACHE[key] = _build_kernel(key)
    return _NC_CACHE[key]


# ------------------------------------------------------------------- driver
def kernel(ts, cd, cc, cb, ca, Wi0, bi0, Wi1, bi1, Wi2, bi2,
           Wf0, bf0, Wf1, bf1, Wf2, bf2, Wl, bl):
    import ml_dtypes
    from concourse.bass_utils import run_bass_kernel_spmd

    f32 = np.float32
    ts, cd, cc, cb, ca = (np.asarray(x, f32) for x in (ts, cd, cc, cb, ca))
    dX9, y0 = _host_precompute(ts, cd, cc, cb, ca, Wi0, bi0, Wi1, bi1, Wi2, bi2)

    Wf0, Wf1, Wf2, Wl = (np.asarray(x, f32) for x in (Wf0, Wf1, Wf2, Wl))
    bf0, bf1, bf2, bl = (np.asarray(x, f32) for x in (bf0, bf1, bf2, bl))

    shared = {
        "w0t": np.ascontiguousarray(Wf0.T),              # (H, W)
        "w1t": np.ascontiguousarray(Wf1.T),              # (W, W)
        "w2t": np.ascontiguousarray(Wf2.T),              # (W, H*D)
        "wlt": np.ascontiguousarray(Wl[0][:, None]),     # (H, 1)
        "b0": np.ascontiguousarray(bf0[:, None]),
        "b1": np.ascontiguousarray(bf1[:, None]),
        "b2": np.ascontiguousarray(bf2[None, :]),
        "ones1": np.ones((1, S), f32),
        "ident": np.eye(S, dtype=f32),
    }

    nc = _get_nc(float(bl[0]))
    in_maps = []
    for c in range(NCORES):
        sl = slice(c * S, (c + 1) * S)
        m = dict(shared)
        m["dx9"] = np.ascontiguousarray(
            dX9[sl].reshape(S, T * D9)).astype(ml_dtypes.bfloat16)
        m["y0t"] = np.ascontiguousarray(y0[sl].T)        # (H, S)
        in_maps.append(m)

    res = run_bass_kernel_spmd(nc, in_maps, core_ids=list(range(NCORES)))
    global LAST_RESULTS
    LAST_RESULTS = res
    out = np.concatenate([res.results[c]["out"] for c in range(NCORES)], axis=0)
    return out.astype(f32)



# revision 13
# speedup vs baseline: 2.1380x; 2.1380x over previous
"""Neural CDE kernel for Trainium2 (8 NeuronCores, data-parallel over batch).

Problem shapes (hardcoded per contract): B=512, T=1024, D=8, H=64, W=128.

The per-call wall time is dominated by host->device transfer over the
axon tunnel (~30 MB/s, ~90 ms/dispatch), so the driver is built around
minimizing per-call bytes:
  - the MLP weights are baked into the NEFF as Const tensors (shipped
    once at model load, not per call);
  - the spline derivative stream is shipped as fp8 e4m3 (1 B/elem),
    prescaled by a power of two 2^p chosen from its absmax; the descale
    2^-p is applied (exactly) on device during the bf16 upcast;
  - only 8 derivative columns go over the wire; the 9th (the
    tanh-rewrite constant term dt*sum_d dX) is derived on device;
  - the output is bf16.

Host side: knot index / frac from ts (exact fp32 accumulation
semantics), spline derivative dX folded with -2*dt*2^p, fp8 encode via
a bf16-bits -> e4m3 LUT, and the initial MLP y0.

Device side (per core, 64 samples, scan unrolled x16 in a hw loop):
  p1 = Wf0 @ y            (PE, f32)
  h1 = ln(1 + exp(p1+b0)) (ACT Exp + Ln(bias=1)) -> bf16
  p2 = Wf1 @ h1           (PE, bf16)
  h2 = ln(1 + exp(p2+b1)) (ACT) -> bf16
  z  = h2^T W2T + b2      (PE, + K=1 ones-matmul for the bias)
  S  = sigmoid(-2z) = exp(-ln(1+exp(2z)))          (ACT x3)
  q[s,h] = sum_d S9[s,(h,d)] * dx9[s,k,d]          (DVE mul + grouped reduce)
           where S9 has a constant-1 column at d=8, dx9[...,d<8] =
           -2*dt*2^p*dX (fp8-upcast), dx9[...,8] = dt*2^p*sum_d dX
  y += (q^T) * 2^-p       (PE transpose against eye*2^-p + DVE add)
  ro[:,k] = y^T @ Wl      (PE, N=1 matmul into accumulating PSUM bank)
Final: sigmoid via the same exp/ln chain, bf16 DMA out.
"""

import hashlib

import numpy as np

B, T, D, H, W = 512, 1024, 8, 64, 128
NCORES = 8
S = B // NCORES  # samples per core = 64
D9 = D + 1       # padded derivative cols

_F32 = np.float32


# ----------------------------------------------------------------- fp8 encode
_LUT_CACHE = {}


def _e4m3_lut(p):
    """uint8 LUT over bf16 bit patterns for x -> e4m3(x * 2^p)."""
    import ml_dtypes

    lut = _LUT_CACHE.get(p)
    if lut is None:
        with np.errstate(all="ignore"):
            vals = np.arange(65536, dtype=np.uint16).view(ml_dtypes.bfloat16)
            scaled = vals.astype(_F32) * _F32(2.0 ** p)
            lut = scaled.astype(ml_dtypes.float8_e4m3).view(np.uint8)
        _LUT_CACHE[p] = lut
    return lut


# ----------------------------------------------------------------- host math
def _knots(ts_row):
    """Index/frac series for one uniform-grid row, matching the oracle's
    fp32 accumulation + searchsorted('right') semantics."""
    Tn = ts_row.shape[0]
    dt = _F32(ts_row[1] - ts_row[0])
    incs = np.full(Tn, dt, _F32)
    incs[0] = ts_row[0]
    t0 = np.cumsum(incs, dtype=_F32)
    i1 = np.clip(np.searchsorted(ts_row, t0, side="right") - 1, 0, Tn - 2)
    frac = (t0 - ts_row[i1]).astype(_F32)
    return i1.astype(np.int64), frac, dt


def _host_precompute(ts, cd, cc, cb, ca, Wi0, bi0, Wi1, bi1, Wi2, bi2):
    """Returns dX8 = -2*dt*dX as (B, T, D) f32, and y0 as (B, H) f32."""
    f32 = _F32
    ts = np.asarray(ts, f32)
    cd = np.asarray(cd, f32)
    cc = np.asarray(cc, f32)
    cb = np.asarray(cb, f32)

    dX8 = np.empty((B, T, D), f32)
    if bool((ts[:1] == ts).all()):
        # fast path: one shared knot series; gather is a unit shift except
        # where fp32 drift moved the knot, fixed up column-wise after.
        i1, frac, dt = _knots(ts[0])
        s = f32(-2.0) * dt
        cB = (s * f32(2.0) * frac).astype(f32)           # cc coefficient
        cC = (s * f32(3.0) * frac * frac).astype(f32)    # cd coefficient
        exp_idx = np.maximum(np.arange(T) - 1, 0)
        fix = np.where(i1 != exp_idx)[0]

        v = dX8[:, 1:]
        np.multiply(cd[:, : T - 1], cC[1:][None, :, None], out=v)
        v += cc[:, : T - 1] * cB[1:][None, :, None]
        v += cb[:, : T - 1] * s
        dX8[:, 0] = s * cb[:, 0] + cB[0] * cc[:, 0] + cC[0] * cd[:, 0]
        if fix.size:
            fi = i1[fix]
            dX8[:, fix] = (
                s * cb[:, fi]
                + cB[fix][None, :, None] * cc[:, fi]
                + cC[fix][None, :, None] * cd[:, fi]
            )
    else:
        # general path: per-row searchsorted (oracle semantics)
        dtv = (ts[:, 1] - ts[:, 0]).astype(f32)
        incs = np.concatenate([ts[:, :1], np.tile(dtv[:, None], (1, T - 1))], 1)
        t0 = np.cumsum(incs, axis=1, dtype=f32)
        idx = np.empty((B, T), np.int64)
        for b in range(B):
            idx[b] = np.searchsorted(ts[b], t0[b], side="right") - 1
        idx = np.clip(idx, 0, T - 2)
        frac = (t0 - np.take_along_axis(ts, idx, axis=1)).astype(f32)
        rows = np.arange(B)[:, None]
        fr = frac[:, :, None]
        dX = (cb[rows, idx] + fr * (f32(2.0) * cc[rows, idx]
                                    + f32(3.0) * fr * cd[rows, idx])).astype(f32)
        np.multiply(dX, (f32(-2.0) * dtv)[:, None, None], out=dX8)

    # initial MLP (relu hidden)
    a0 = np.asarray(ca, f32)[:, 0, :]
    hh = np.maximum(a0 @ np.asarray(Wi0, f32).T + np.asarray(bi0, f32), 0)
    hh = np.maximum(hh @ np.asarray(Wi1, f32).T + np.asarray(bi1, f32), 0)
    y0 = (hh @ np.asarray(Wi2, f32).T + np.asarray(bi2, f32)).astype(f32)
    return dX8, y0


# --------------------------------------------------------------- bass kernel
def _build_kernel(bl_val, w0t, w1t_bf, w2t_bf, wlt, b0, b1, b2, ones1):
    import concourse.bass as bass
    import concourse.bacc as bacc
    import concourse.mybir as mybir
    from concourse.tile import TileContext

    f32 = mybir.dt.float32
    bf16 = mybir.dt.bfloat16
    f8 = mybir.dt.float8e4
    AF = mybir.ActivationFunctionType
    ALU = mybir.AluOpType

    nc = bacc.Bacc("TRN2")

    # per-call inputs / outputs
    d_dx8 = nc.dram_tensor("dx8", [S, T * D], f8, kind="ExternalInput")
    d_scl = nc.dram_tensor("scl", [S, 1], f32, kind="ExternalInput")    # 2^-p
    d_y0t = nc.dram_tensor("y0t", [H, S], f32, kind="ExternalInput")
    d_out = nc.dram_tensor("out", [S, T], bf16, kind="ExternalOutput")

    # weights baked into the NEFF (loaded to HBM once, not shipped per call)
    c_w0t = nc.inline_tensor(w0t, name="cw0t")        # (H, W)  f32
    c_w1t = nc.inline_tensor(w1t_bf, name="cw1t")     # (W, W)  bf16
    c_w2t = nc.inline_tensor(w2t_bf, name="cw2t")     # (W, H*D) bf16
    c_wlt = nc.inline_tensor(wlt, name="cwlt")        # (H, 1)  f32
    c_b0 = nc.inline_tensor(b0, name="cb0")           # (W, 1)  f32
    c_b1 = nc.inline_tensor(b1, name="cb1")           # (W, 1)  f32
    c_b2 = nc.inline_tensor(b2, name="cb2")           # (1, H*D) bf16
    c_ones = nc.inline_tensor(ones1, name="cones")    # (1, S)  bf16
    # PE transpose needs a true permutation matrix (values are routing,
    # not multiplied), so the identity is exact and baked
    c_eye = nc.inline_tensor(np.eye(S, dtype=np.float32), name="ceye")

    UNROLL = 16
    assert T % UNROLL == 0

    with TileContext(nc) as tc:
        with (
            tc.tile_pool(name="const", bufs=1) as cpool,
            tc.tile_pool(name="state", bufs=1) as spool,
            tc.tile_pool(name="work", bufs=2) as wpool,
            tc.tile_pool(name="ps", bufs=2, space="PSUM") as ppool,
            tc.tile_pool(name="ps1", bufs=1, space="PSUM") as p1pool,
        ):
            # constants
            w0t_t = cpool.tile([H, W], f32, tag="w0t")
            w1t_t = cpool.tile([W, W], bf16, tag="w1t")
            w2t_t = cpool.tile([W, H * D], bf16, tag="w2t")
            wlt_t = cpool.tile([H, 1], f32, tag="wlt")
            b0_t = cpool.tile([W, 1], f32, tag="b0")
            b1_t = cpool.tile([W, 1], f32, tag="b1")
            # bias matmul operands in bf16: both matmuls accumulating into
            # the vf PSUM group must share the PE dtype mode
            b2_t = cpool.tile([1, H * D], bf16, tag="b2")
            ones_t = cpool.tile([1, S], bf16, tag="ones1")
            eye_t = cpool.tile([S, S], f32, tag="eye")
            scl_t = cpool.tile([S, 1], f32, tag="scl")
            dx8_t = cpool.tile([S, T * D], f8, tag="dx8")
            dx9 = cpool.tile([S, T * D9], bf16, tag="dx9")
            sum8 = cpool.tile([S, T], f32, tag="sum8")
            for dst, src in [
                (w0t_t, c_w0t), (w1t_t, c_w1t), (w2t_t, c_w2t), (wlt_t, c_wlt),
                (b0_t, c_b0), (b1_t, c_b1), (b2_t, c_b2), (ones_t, c_ones),
                (eye_t, c_eye), (scl_t, d_scl), (dx8_t, d_dx8),
            ]:
                nc.gpsimd.dma_start(dst[:], src[:])

            # state
            y = spool.tile([H, S], f32, tag="y")  # (h, s)
            nc.gpsimd.dma_start(y[:], d_y0t[:])
            # S9 double buffer, const-1 column at d=8
            s9 = [
                spool.tile([S, H * D9], bf16, tag=f"s9_{i}", name=f"s9_{i}")
                for i in range(2)
            ]
            for t_ in s9:
                v = t_[:].rearrange("s (h d) -> s h d", d=D9)
                nc.vector.memset(v[:, :, D : D + 1], 1.0)

            # expand dx8 (fp8) -> dx9 (bf16): upcast + descale by 2^-p in one
            # DVE pass (per-partition scalar), then derive the 9th column
            dx8v = dx8_t[:].rearrange("s (t d) -> s t d", d=D)
            dx9v = dx9[:].rearrange("s (t d) -> s t d", d=D9)
            nc.vector.tensor_scalar(
                dx9v[:, :, 0:D], dx8v, scl_t[:], None, ALU.mult
            )
            nc.vector.tensor_reduce(
                sum8[:], dx9v[:, :, 0:D], axis=mybir.AxisListType.X, op=ALU.add
            )
            sum8v = sum8[:].rearrange("s (t o) -> s t o", o=1)
            nc.vector.tensor_scalar_mul(dx9v[:, :, D : D + 1], sum8v, -0.5)

            ro_sb = spool.tile([S, T], f32, tag="ro_sb")
            ro_ps = p1pool.tile([S, UNROLL], f32, tag="ro_ps")

            # Constants settle before any compute touches them: a matmul
            # (S3_LW struct) cannot carry multiple HWDGE sem waits.
            tc.strict_bb_all_engine_barrier()

            with tc.For_i(0, T // UNROLL, 1) as iv:
              ibase = iv * (UNROLL * D9)
              for j in range(UNROLL):
                s9k = s9[j % 2]
                # ---- mm1: p1 = Wf0 @ y  -> (W, S)
                p1 = ppool.tile([W, S], f32, tag="p12")
                nc.tensor.matmul(p1[:], w0t_t[:], y[:], start=True, stop=True)
                # ---- softplus 1 (bias b0 folded into Exp) -> bf16
                u1 = wpool.tile([W, S], f32, tag="u1")
                h1 = wpool.tile([W, S], bf16, tag="h1")
                nc.scalar.activation(u1[:], p1[:], AF.Exp, bias=b0_t[:])
                nc.scalar.activation(h1[:], u1[:], AF.Ln, bias=1.0)
                # ---- mm2 (bf16)
                p2 = ppool.tile([W, S], f32, tag="p12")
                nc.tensor.matmul(p2[:], w1t_t[:], h1[:], start=True, stop=True)
                u2 = wpool.tile([W, S], f32, tag="u2")
                h2 = wpool.tile([W, S], bf16, tag="h2")
                nc.scalar.activation(u2[:], p2[:], AF.Exp, bias=b1_t[:])
                nc.scalar.activation(h2[:], u2[:], AF.Ln, bias=1.0)
                # ---- mm3: z = h2^T W2T + b2 -> (S, H*D)
                vf = ppool.tile([S, H * D], f32, tag="vf")
                nc.tensor.matmul(vf[:], ones_t[:], b2_t[:], start=True, stop=False)
                nc.tensor.matmul(vf[:], h2[:], w2t_t[:], start=False, stop=True)
                # ---- S = sigmoid(-2z) = exp(-ln(1+exp(2z)))
                e2 = wpool.tile([S, H * D], f32, tag="e2")
                l2 = wpool.tile([S, H * D], f32, tag="l2")
                nc.scalar.activation(e2[:], vf[:], AF.Exp, scale=2.0)
                nc.scalar.activation(l2[:], e2[:], AF.Ln, bias=1.0)
                s9v = s9k[:].rearrange("s (h d) -> s h d", d=D9)
                l2v = l2[:].rearrange("s (h d) -> s h d", d=D)
                nc.scalar.activation(s9v[:, :, 0:D], l2v, AF.Exp, scale=-1.0)
                # ---- q[s,h] = sum_d S9 * dx9  (broadcast dx over h)
                m1 = wpool.tile([S, H * D9], bf16, tag="m1")
                dxk = dx9[:, bass.ds(ibase + j * D9, D9)]
                dxb = dxk.rearrange("s (o d) -> s o d", o=1)
                m1v = m1[:].rearrange("s (h d) -> s h d", d=D9)
                s9vv = s9k[:].rearrange("s (h d) -> s h d", d=D9)
                in0b, in1b = bass.broadcast_tensor_aps(s9vv, dxb)
                nc.vector.tensor_tensor(m1v, in0b, in1b, ALU.mult)
                q = wpool.tile([S, H], f32, tag="q")
                nc.vector.tensor_reduce(
                    q[:], m1v, axis=mybir.AxisListType.X, op=ALU.add
                )
                # ---- y += q^T
                qt = ppool.tile([H, S], f32, tag="qt")
                nc.tensor.transpose(qt[:], q[:], eye_t[:])
                nc.vector.tensor_tensor(y[:], y[:], qt[:], ALU.add)
                # ---- readout column
                nc.tensor.matmul(
                    ro_ps[:, j : j + 1], y[:], wlt_t[:], start=True, stop=True
                )
                if j == UNROLL - 1:
                    nc.vector.tensor_copy(
                        ro_sb[:, bass.ds(iv * UNROLL, UNROLL)], ro_ps[:]
                    )

            # ---- final sigmoid(v + bl) = exp(-ln(1+exp(-v-bl))), bf16 out
            eo = spool.tile([S, T], f32, tag="eo")
            eo2 = spool.tile([S, T], bf16, tag="eo2")
            nc.scalar.activation(eo[:], ro_sb[:], AF.Exp, scale=-1.0,
                                 bias=float(-bl_val))
            nc.scalar.activation(eo[:], eo[:], AF.Ln, bias=1.0)
            nc.scalar.activation(eo2[:], eo[:], AF.Exp, scale=-1.0)
            nc.sync.dma_start(d_out[:], eo2[:])

    nc.compile()
    return nc


_NC_CACHE = {}
LAST_RESULTS = None


def _get_nc(bl_val, consts):
    h = hashlib.sha1(repr(float(bl_val)).encode())
    for a in consts:
        h.update(a.tobytes())
    key = h.hexdigest()
    if key not in _NC_CACHE:
        _NC_CACHE[key] = _build_kernel(float(bl_val), *consts)
    return _NC_CACHE[key]


# ------------------------------------------------------------------- driver
def kernel(ts, cd, cc, cb, ca, Wi0, bi0, Wi1, bi1, Wi2, bi2,
           Wf0, bf0, Wf1, bf1, Wf2, bf2, Wl, bl):
    import ml_dtypes
    from concourse.bass_utils import run_bass_kernel_spmd

    f32 = _F32
    dX8, y0 = _host_precompute(ts, cd, cc, cb, ca, Wi0, bi0, Wi1, bi1, Wi2, bi2)

    # power-of-two prescale so the fp8 payload uses the e4m3 range
    absmax = float(max(dX8.max(), -dX8.min(), 1e-30))
    p = int(np.floor(np.log2(224.0 / absmax)))
    p = max(min(p, 120), -120)
    lut = _e4m3_lut(p)

    Wf0, Wf1, Wf2, Wl = (np.asarray(x, f32) for x in (Wf0, Wf1, Wf2, Wl))
    bf0_, bf1_, bf2_ = (np.asarray(x, f32) for x in (bf0, bf1, bf2))
    bl_val = float(np.asarray(bl, f32).reshape(-1)[0])
    consts = (
        np.ascontiguousarray(Wf0.T),                          # w0t (H, W) f32
        np.ascontiguousarray(Wf1.T).astype(ml_dtypes.bfloat16),   # w1t bf16
        np.ascontiguousarray(Wf2.T).astype(ml_dtypes.bfloat16),   # w2t bf16
        np.ascontiguousarray(Wl[0][:, None]),                 # wlt (H, 1)
        np.ascontiguousarray(bf0_[:, None]),
        np.ascontiguousarray(bf1_[:, None]),
        np.ascontiguousarray(bf2_[None, :]).astype(ml_dtypes.bfloat16),
        np.ones((1, S), f32).astype(ml_dtypes.bfloat16),
    )
    nc = _get_nc(bl_val, consts)

    scl = np.full((S, 1), f32(2.0 ** (-p)), f32)
    dx8_bits = dX8.astype(ml_dtypes.bfloat16).view(np.uint16)
    in_maps = []
    for c in range(NCORES):
        sl = slice(c * S, (c + 1) * S)
        in_maps.append({
            "dx8": lut[dx8_bits[sl]].reshape(S, T * D)
                   .view(ml_dtypes.float8_e4m3),
            "scl": scl,
            "y0t": np.ascontiguousarray(y0[sl].T),            # (H, S)
        })

    res = run_bass_kernel_spmd(nc, in_maps, core_ids=list(range(NCORES)))
    global LAST_RESULTS
    LAST_RESULTS = res
    out = np.concatenate([res.results[c]["out"] for c in range(NCORES)], axis=0)
    return out.astype(f32)


# revision 16
# speedup vs baseline: 5.1772x; 2.4215x over previous
"""Neural CDE kernel for Trainium2 (8 NeuronCores, data-parallel over batch).

Problem shapes (hardcoded per contract): B=512, T=1024, D=8, H=64, W=128.

The per-call wall time is dominated by host->device transfer over the
axon tunnel (~30 MB/s, ~90 ms/dispatch), so the driver is built around
minimizing per-call bytes:
  - the MLP weights are baked into the NEFF as Const tensors (shipped
    once at model load, not per call);
  - the spline derivative stream is shipped as fp8 e4m3 (1 B/elem),
    prescaled by a power of two 2^p chosen from its absmax; the descale
    2^-p is applied (exactly) on device during the bf16 upcast;
  - only 8 derivative columns go over the wire; the 9th (the
    tanh-rewrite constant term dt*sum_d dX) is derived on device;
  - the output is bf16.

Host side: knot index / frac from ts (exact fp32 accumulation
semantics), spline derivative dX folded with -2*dt*2^p, fp8 encode via
a bf16-bits -> e4m3 LUT, and the initial MLP y0.

Device side (per core, 64 samples, scan unrolled x16 in a hw loop):
  p1 = Wf0 @ y            (PE, f32)
  h1 = ln(1 + exp(p1+b0)) (ACT Exp + Ln(bias=1)) -> bf16
  p2 = Wf1 @ h1           (PE, bf16)
  h2 = ln(1 + exp(p2+b1)) (ACT) -> bf16
  z  = h2^T W2T + b2      (PE, + K=1 ones-matmul for the bias)
  S  = sigmoid(-2z) = exp(-ln(1+exp(2z)))          (ACT x3)
  q[s,h] = sum_d S9[s,(h,d)] * dx9[s,k,d]          (DVE mul + grouped reduce)
           where S9 has a constant-1 column at d=8, dx9[...,d<8] =
           -2*dt*2^p*dX (fp8-upcast), dx9[...,8] = dt*2^p*sum_d dX
  y += (q^T) * 2^-p       (PE transpose against eye*2^-p + DVE add)
  ro[:,k] = y^T @ Wl      (PE, N=1 matmul into accumulating PSUM bank)
Final: sigmoid via the same exp/ln chain, bf16 DMA out.
"""

import hashlib

import numpy as np

B, T, D, H, W = 512, 1024, 8, 64, 128
NCORES = 8
S = B // NCORES  # samples per core = 64
D9 = D + 1       # padded derivative cols

_F32 = np.float32


# ----------------------------------------------------------------- fp8 encode
_LUT_CACHE = {}


def _e4m3_lut(p):
    """uint8 LUT over bf16 bit patterns for x -> e4m3(x * 2^p)."""
    import ml_dtypes

    lut = _LUT_CACHE.get(p)
    if lut is None:
        with np.errstate(all="ignore"):
            vals = np.arange(65536, dtype=np.uint16).view(ml_dtypes.bfloat16)
            scaled = vals.astype(_F32) * _F32(2.0 ** p)
            lut = scaled.astype(ml_dtypes.float8_e4m3).view(np.uint8)
        _LUT_CACHE[p] = lut
    return lut


# ----------------------------------------------------------------- host math
def _knots(ts_row):
    """Index/frac series for one uniform-grid row, matching the oracle's
    fp32 accumulation + searchsorted('right') semantics."""
    Tn = ts_row.shape[0]
    dt = _F32(ts_row[1] - ts_row[0])
    incs = np.full(Tn, dt, _F32)
    incs[0] = ts_row[0]
    t0 = np.cumsum(incs, dtype=_F32)
    i1 = np.clip(np.searchsorted(ts_row, t0, side="right") - 1, 0, Tn - 2)
    frac = (t0 - ts_row[i1]).astype(_F32)
    return i1.astype(np.int64), frac, dt


def _host_precompute(ts, cd, cc, cb, ca, Wi0, bi0, Wi1, bi1, Wi2, bi2):
    """Returns dX8 = -2*dt*dX as (B, T, D) f32, and y0 as (B, H) f32."""
    f32 = _F32
    ts = np.asarray(ts, f32)
    cd = np.asarray(cd, f32)
    cc = np.asarray(cc, f32)
    cb = np.asarray(cb, f32)

    dX8 = np.empty((B, T, D), f32)
    if bool((ts[:1] == ts).all()):
        # fast path: one shared knot series; gather is a unit shift except
        # where fp32 drift moved the knot, fixed up column-wise after.
        i1, frac, dt = _knots(ts[0])
        s = f32(-2.0) * dt
        cB = (s * f32(2.0) * frac).astype(f32)           # cc coefficient
        cC = (s * f32(3.0) * frac * frac).astype(f32)    # cd coefficient
        exp_idx = np.maximum(np.arange(T) - 1, 0)
        fix = np.where(i1 != exp_idx)[0]

        v = dX8[:, 1:]
        np.multiply(cd[:, : T - 1], cC[1:][None, :, None], out=v)
        v += cc[:, : T - 1] * cB[1:][None, :, None]
        v += cb[:, : T - 1] * s
        dX8[:, 0] = s * cb[:, 0] + cB[0] * cc[:, 0] + cC[0] * cd[:, 0]
        if fix.size:
            fi = i1[fix]
            dX8[:, fix] = (
                s * cb[:, fi]
                + cB[fix][None, :, None] * cc[:, fi]
                + cC[fix][None, :, None] * cd[:, fi]
            )
    else:
        # general path: per-row searchsorted (oracle semantics)
        dtv = (ts[:, 1] - ts[:, 0]).astype(f32)
        incs = np.concatenate([ts[:, :1], np.tile(dtv[:, None], (1, T - 1))], 1)
        t0 = np.cumsum(incs, axis=1, dtype=f32)
        idx = np.empty((B, T), np.int64)
        for b in range(B):
            idx[b] = np.searchsorted(ts[b], t0[b], side="right") - 1
        idx = np.clip(idx, 0, T - 2)
        frac = (t0 - np.take_along_axis(ts, idx, axis=1)).astype(f32)
        rows = np.arange(B)[:, None]
        fr = frac[:, :, None]
        dX = (cb[rows, idx] + fr * (f32(2.0) * cc[rows, idx]
                                    + f32(3.0) * fr * cd[rows, idx])).astype(f32)
        np.multiply(dX, (f32(-2.0) * dtv)[:, None, None], out=dX8)

    # initial MLP (relu hidden)
    a0 = np.asarray(ca, f32)[:, 0, :]
    hh = np.maximum(a0 @ np.asarray(Wi0, f32).T + np.asarray(bi0, f32), 0)
    hh = np.maximum(hh @ np.asarray(Wi1, f32).T + np.asarray(bi1, f32), 0)
    y0 = (hh @ np.asarray(Wi2, f32).T + np.asarray(bi2, f32)).astype(f32)
    return dX8, y0


# --------------------------------------------------------------- bass kernel
def _build_kernel(bl_val, w0t, w1t_bf, w2t_bf, wlt, b0, b1, b2, ones1):
    import concourse.bass as bass
    import concourse.bacc as bacc
    import concourse.mybir as mybir
    from concourse.tile import TileContext

    f32 = mybir.dt.float32
    bf16 = mybir.dt.bfloat16
    f8 = mybir.dt.float8e4
    AF = mybir.ActivationFunctionType
    ALU = mybir.AluOpType

    nc = bacc.Bacc("TRN2")

    # per-call inputs / outputs
    d_dx8 = nc.dram_tensor("dx8", [S, T * D], f8, kind="ExternalInput")
    d_scl = nc.dram_tensor("scl", [S, 1], f32, kind="ExternalInput")    # 2^-p
    d_y0t = nc.dram_tensor("y0t", [H, S], f32, kind="ExternalInput")
    d_out = nc.dram_tensor("out", [S, T], bf16, kind="ExternalOutput")

    # weights baked into the NEFF (loaded to HBM once, not shipped per call)
    c_w0t = nc.inline_tensor(w0t, name="cw0t")        # (H, W)  f32
    c_w1t = nc.inline_tensor(w1t_bf, name="cw1t")     # (W, W)  bf16
    c_w2t = nc.inline_tensor(w2t_bf, name="cw2t")     # (W, H*D) bf16
    c_wlt = nc.inline_tensor(wlt, name="cwlt")        # (H, 1)  f32
    c_b0 = nc.inline_tensor(b0, name="cb0")           # (W, 1)  f32
    c_b1 = nc.inline_tensor(b1, name="cb1")           # (W, 1)  f32
    c_b2 = nc.inline_tensor(b2, name="cb2")           # (1, H*D) bf16
    c_ones = nc.inline_tensor(ones1, name="cones")    # (1, S)  bf16
    # PE transpose needs a true permutation matrix (values are routing,
    # not multiplied), so the identity is exact and baked
    c_eye = nc.inline_tensor(np.eye(S, dtype=np.float32), name="ceye")

    UNROLL = 16
    assert T % UNROLL == 0

    with TileContext(nc) as tc:
        with (
            tc.tile_pool(name="const", bufs=1) as cpool,
            tc.tile_pool(name="state", bufs=1) as spool,
            tc.tile_pool(name="work", bufs=2) as wpool,
            tc.tile_pool(name="ps", bufs=2, space="PSUM") as ppool,
            tc.tile_pool(name="ps1", bufs=1, space="PSUM") as p1pool,
        ):
            # constants
            w0t_t = cpool.tile([H, W], f32, tag="w0t")
            w1t_t = cpool.tile([W, W], bf16, tag="w1t")
            w2t_t = cpool.tile([W, H * D], bf16, tag="w2t")
            wlt_t = cpool.tile([H, 1], f32, tag="wlt")
            b0_t = cpool.tile([W, 1], f32, tag="b0")
            b1_t = cpool.tile([W, 1], f32, tag="b1")
            # bias matmul operands in bf16: both matmuls accumulating into
            # the vf PSUM group must share the PE dtype mode
            b2_t = cpool.tile([1, H * D], bf16, tag="b2")
            ones_t = cpool.tile([1, S], bf16, tag="ones1")
            eye_t = cpool.tile([S, S], f32, tag="eye")
            scl_t = cpool.tile([S, 1], f32, tag="scl")
            dx8_t = cpool.tile([S, T * D], f8, tag="dx8")
            dx9 = cpool.tile([S, T * D9], bf16, tag="dx9")
            sum8 = cpool.tile([S, T], f32, tag="sum8")
            for dst, src in [
                (w0t_t, c_w0t), (w1t_t, c_w1t), (w2t_t, c_w2t), (wlt_t, c_wlt),
                (b0_t, c_b0), (b1_t, c_b1), (b2_t, c_b2), (ones_t, c_ones),
                (eye_t, c_eye), (scl_t, d_scl), (dx8_t, d_dx8),
            ]:
                nc.gpsimd.dma_start(dst[:], src[:])

            # state
            y = spool.tile([H, S], f32, tag="y")  # (h, s)
            nc.gpsimd.dma_start(y[:], d_y0t[:])
            # S9 double buffer, const-1 column at d=8
            s9 = [
                spool.tile([S, H * D9], bf16, tag=f"s9_{i}", name=f"s9_{i}")
                for i in range(2)
            ]
            for t_ in s9:
                v = t_[:].rearrange("s (h d) -> s h d", d=D9)
                nc.vector.memset(v[:, :, D : D + 1], 1.0)

            # expand dx8 (fp8) -> dx9 (bf16): upcast + descale by 2^-p in one
            # DVE pass (per-partition scalar), then derive the 9th column
            dx8v = dx8_t[:].rearrange("s (t d) -> s t d", d=D)
            dx9v = dx9[:].rearrange("s (t d) -> s t d", d=D9)
            nc.vector.tensor_scalar(
                dx9v[:, :, 0:D], dx8v, scl_t[:], None, ALU.mult
            )
            nc.vector.tensor_reduce(
                sum8[:], dx9v[:, :, 0:D], axis=mybir.AxisListType.X, op=ALU.add
            )
            sum8v = sum8[:].rearrange("s (t o) -> s t o", o=1)
            nc.vector.tensor_scalar_mul(dx9v[:, :, D : D + 1], sum8v, -0.5)

            ro_sb = spool.tile([S, T], f32, tag="ro_sb")
            ro_ps = p1pool.tile([S, UNROLL], f32, tag="ro_ps")

            # Constants settle before any compute touches them: a matmul
            # (S3_LW struct) cannot carry multiple HWDGE sem waits.
            tc.strict_bb_all_engine_barrier()

            with tc.For_i(0, T // UNROLL, 1) as iv:
              ibase = iv * (UNROLL * D9)
              for j in range(UNROLL):
                s9k = s9[j % 2]
                # ---- mm1: p1 = Wf0 @ y  -> (W, S)
                p1 = ppool.tile([W, S], f32, tag="p12")
                nc.tensor.matmul(p1[:], w0t_t[:], y[:], start=True, stop=True)
                # ---- softplus 1 (bias b0 folded into Exp) -> bf16
                u1 = wpool.tile([W, S], f32, tag="u1")
                h1 = wpool.tile([W, S], bf16, tag="h1")
                nc.scalar.activation(u1[:], p1[:], AF.Exp, bias=b0_t[:])
                nc.scalar.activation(h1[:], u1[:], AF.Ln, bias=1.0)
                # ---- mm2 (bf16)
                p2 = ppool.tile([W, S], f32, tag="p12")
                nc.tensor.matmul(p2[:], w1t_t[:], h1[:], start=True, stop=True)
                u2 = wpool.tile([W, S], f32, tag="u2")
                h2 = wpool.tile([W, S], bf16, tag="h2")
                nc.scalar.activation(u2[:], p2[:], AF.Exp, bias=b1_t[:])
                nc.scalar.activation(h2[:], u2[:], AF.Ln, bias=1.0)
                # ---- mm3: z = h2^T W2T + b2 -> (S, H*D)
                vf = ppool.tile([S, H * D], f32, tag="vf")
                nc.tensor.matmul(vf[:], ones_t[:], b2_t[:], start=True, stop=False)
                nc.tensor.matmul(vf[:], h2[:], w2t_t[:], start=False, stop=True)
                # ---- S = sigmoid(-2z) = exp(-ln(1+exp(2z)))
                e2 = wpool.tile([S, H * D], f32, tag="e2")
                l2 = wpool.tile([S, H * D], f32, tag="l2")
                nc.scalar.activation(e2[:], vf[:], AF.Exp, scale=2.0)
                nc.scalar.activation(l2[:], e2[:], AF.Ln, bias=1.0)
                s9v = s9k[:].rearrange("s (h d) -> s h d", d=D9)
                l2v = l2[:].rearrange("s (h d) -> s h d", d=D)
                nc.scalar.activation(s9v[:, :, 0:D], l2v, AF.Exp, scale=-1.0)
                # ---- q[s,h] = sum_d S9 * dx9  (broadcast dx over h)
                m1 = wpool.tile([S, H * D9], bf16, tag="m1")
                dxk = dx9[:, bass.ds(ibase + j * D9, D9)]
                dxb = dxk.rearrange("s (o d) -> s o d", o=1)
                m1v = m1[:].rearrange("s (h d) -> s h d", d=D9)
                s9vv = s9k[:].rearrange("s (h d) -> s h d", d=D9)
                in0b, in1b = bass.broadcast_tensor_aps(s9vv, dxb)
                nc.vector.tensor_tensor(m1v, in0b, in1b, ALU.mult)
                q = wpool.tile([S, H], f32, tag="q")
                nc.vector.tensor_reduce(
                    q[:], m1v, axis=mybir.AxisListType.X, op=ALU.add
                )
                # ---- y += q^T
                qt = ppool.tile([H, S], f32, tag="qt")
                nc.tensor.transpose(qt[:], q[:], eye_t[:])
                nc.vector.tensor_tensor(y[:], y[:], qt[:], ALU.add)
                # ---- readout column
                nc.tensor.matmul(
                    ro_ps[:, j : j + 1], y[:], wlt_t[:], start=True, stop=True
                )
                if j == UNROLL - 1:
                    nc.vector.tensor_copy(
                        ro_sb[:, bass.ds(iv * UNROLL, UNROLL)], ro_ps[:]
                    )

            # ---- final sigmoid(v + bl) = exp(-ln(1+exp(-v-bl))), bf16 out
            eo = spool.tile([S, T], f32, tag="eo")
            eo2 = spool.tile([S, T], bf16, tag="eo2")
            nc.scalar.activation(eo[:], ro_sb[:], AF.Exp, scale=-1.0,
                                 bias=float(-bl_val))
            nc.scalar.activation(eo[:], eo[:], AF.Ln, bias=1.0)
            nc.scalar.activation(eo2[:], eo[:], AF.Exp, scale=-1.0)
            nc.sync.dma_start(d_out[:], eo2[:])

    nc.compile()
    return nc


_NC_CACHE = {}
_RUN_CACHE = {}
LAST_RESULTS = None


def _spmd_runner(nc):
    """PJRT runner for `nc` on 8 cores, matching run_bass_via_pjrt's
    lowering but with the donated output slot kept device-resident: the
    kernel writes every element of `out`, so call N+1 can donate call N's
    output buffer instead of shipping fresh zeros host->device each call.
    """
    import jax
    import concourse.mybir as mybir
    from jax.sharding import Mesh, NamedSharding, PartitionSpec
    from jax.experimental.shard_map import shard_map
    from concourse import bass2jax

    key = id(nc)
    if key in _RUN_CACHE:
        return _RUN_CACHE[key]

    bass2jax.install_neuronx_cc_hook()

    partition_name = (
        nc.partition_id_tensor.name if nc.partition_id_tensor else None
    )
    in_names, out_names, out_avals = [], [], []
    for alloc in nc.m.functions[0].allocations:
        if not isinstance(alloc, mybir.MemoryLocationSet):
            continue
        name = alloc.memorylocations[0].name
        if alloc.kind == "ExternalInput":
            if name != partition_name:
                in_names.append(name)
        elif alloc.kind == "ExternalOutput":
            shape = tuple(alloc.tensor_shape)
            dtype = mybir.dt.np(alloc.dtype)
            out_avals.append(jax.core.ShapedArray(shape, dtype))
            out_names.append(name)
    n_params = len(in_names)
    all_names = in_names + out_names
    if partition_name is not None:
        all_names = all_names + [partition_name]

    def _body(*args):
        operands = list(args)
        if partition_name is not None:
            operands.append(bass2jax.partition_id_tensor())
        return tuple(_bind(*operands))

    def _bind(*operands):
        return bass2jax._bass_exec_p.bind(
            *operands,
            out_avals=tuple(out_avals),
            in_names=tuple(all_names),
            out_names=tuple(out_names),
            lowering_input_output_aliases=(),
            sim_require_finite=True,
            sim_require_nnan=True,
            nc=nc,
        )

    devices = jax.devices()[:NCORES]
    if len(devices) < NCORES:
        raise RuntimeError("need 8 devices")
    mesh = Mesh(np.asarray(devices), ("core",))
    nin = n_params + len(out_names)
    fn = jax.jit(
        shard_map(
            _body,
            mesh=mesh,
            in_specs=(PartitionSpec("core"),) * nin,
            out_specs=(PartitionSpec("core"),) * len(out_names),
            check_rep=False,
        ),
        donate_argnums=tuple(range(n_params, nin)),
        keep_unused=True,
    )
    sharding = NamedSharding(mesh, PartitionSpec("core"))
    state = {
        "fn": fn,
        "in_names": in_names,
        "out_avals": out_avals,
        "sharding": sharding,
        "slots": None,
    }
    _RUN_CACHE[key] = state
    return state


def _run_spmd(nc, global_in):
    """global_in: dict name -> global (NCORES*dim0, ...) np array."""
    import jax

    st = _spmd_runner(nc)
    if st["slots"] is None:
        st["slots"] = [
            jax.device_put(
                np.zeros((NCORES * a.shape[0], *a.shape[1:]), a.dtype),
                st["sharding"],
            )
            for a in st["out_avals"]
        ]
    args = [global_in[n] for n in st["in_names"]]
    outs = st["fn"](*args, *st["slots"])
    st["slots"] = list(outs)
    return [np.asarray(o) for o in outs]


def _get_nc(bl_val, consts):
    h = hashlib.sha1(repr(float(bl_val)).encode())
    for a in consts:
        h.update(a.tobytes())
    key = h.hexdigest()
    if key not in _NC_CACHE:
        _NC_CACHE[key] = _build_kernel(float(bl_val), *consts)
    return _NC_CACHE[key]


# ------------------------------------------------------------------- driver
def kernel(ts, cd, cc, cb, ca, Wi0, bi0, Wi1, bi1, Wi2, bi2,
           Wf0, bf0, Wf1, bf1, Wf2, bf2, Wl, bl):
    import ml_dtypes

    f32 = _F32
    dX8, y0 = _host_precompute(ts, cd, cc, cb, ca, Wi0, bi0, Wi1, bi1, Wi2, bi2)

    # power-of-two prescale so the fp8 payload uses the e4m3 range
    absmax = float(max(dX8.max(), -dX8.min(), 1e-30))
    p = int(np.floor(np.log2(224.0 / absmax)))
    p = max(min(p, 120), -120)
    lut = _e4m3_lut(p)

    Wf0, Wf1, Wf2, Wl = (np.asarray(x, f32) for x in (Wf0, Wf1, Wf2, Wl))
    bf0_, bf1_, bf2_ = (np.asarray(x, f32) for x in (bf0, bf1, bf2))
    bl_val = float(np.asarray(bl, f32).reshape(-1)[0])
    consts = (
        np.ascontiguousarray(Wf0.T),                          # w0t (H, W) f32
        np.ascontiguousarray(Wf1.T).astype(ml_dtypes.bfloat16),   # w1t bf16
        np.ascontiguousarray(Wf2.T).astype(ml_dtypes.bfloat16),   # w2t bf16
        np.ascontiguousarray(Wl[0][:, None]),                 # wlt (H, 1)
        np.ascontiguousarray(bf0_[:, None]),
        np.ascontiguousarray(bf1_[:, None]),
        np.ascontiguousarray(bf2_[None, :]).astype(ml_dtypes.bfloat16),
        np.ones((1, S), f32).astype(ml_dtypes.bfloat16),
    )
    nc = _get_nc(bl_val, consts)

    global LAST_RESULTS
    dx8_bits = dX8.astype(ml_dtypes.bfloat16).view(np.uint16)
    dx8 = lut[dx8_bits].reshape(B, T * D).view(ml_dtypes.float8_e4m3)
    # per-core (H, S) blocks of y0^T stacked -> (NCORES*H, S)
    y0t = np.ascontiguousarray(
        y0.reshape(NCORES, S, H).transpose(0, 2, 1).reshape(NCORES * H, S))
    scl = np.full((B, 1), f32(2.0 ** (-p)), f32)
    try:
        outs = _run_spmd(nc, {"dx8": dx8, "scl": scl, "y0t": y0t})
        LAST_RESULTS = None
        return outs[0].astype(f32)
    except Exception:
        from concourse.bass_utils import run_bass_kernel_spmd

        in_maps = []
        for c in range(NCORES):
            sl = slice(c * S, (c + 1) * S)
            in_maps.append({
                "dx8": np.ascontiguousarray(dx8[sl]),
                "scl": scl[sl],
                "y0t": np.ascontiguousarray(y0t[c * H : (c + 1) * H]),
            })
        res = run_bass_kernel_spmd(nc, in_maps, core_ids=list(range(NCORES)))
        LAST_RESULTS = res
        out = np.concatenate(
            [res.results[c]["out"] for c in range(NCORES)], axis=0)
        return out.astype(f32)
